# revision 10
# baseline (speedup 1.0000x reference)
"""Trainium2 Bass kernel for CrossAttentionComplexQ.

Shapes: q_real (64, 8, 256), kv (64, 4096, 512) -> out (64, 8, 256).

Math (per batch b):
    q  = complex-rotate(q_real, freq)           # rope-style pairwise rotation
    qn = LayerNorm(q) * ln_w + ln_b
    k  = kv @ Wk.T ; v = kv @ Wv.T
    out = ((qn @ k.T) / 16) @ v
Since there is no softmax the whole chain is linear in kv, so per batch:
    out = qk' @ G @ Wv.T    with qk' = qn @ Wk / 16 (8, 512)
                            and  G = kv.T @ kv      (512, 512 Gram matrix)
G only ever contracts over the sequence dim s, so kv is consumed in its
natural [s-partition, d-free] SBUF layout -- no transposes of the big
tensor. G is symmetric: only the 10 upper-triangle 128x128 blocks are
computed; mirror blocks come from XBAR DMA transposes (off the PE).

Perf structure:
  - kv cast to fp8 e4m3 on host; Gram matmuls run DoubleRow perf mode
    (k=256 per instruction). PSUM-drain-limited at ~1 out-col/cycle.
  - DMA issue cost (~0.6us/DMA on an engine queue) is spread over the
    three DMA-capable queues: kv on sync, weights + mirror-transposes +
    out on scalar(+sync), q-stage on gpsimd.
  - Software pipelining: batch b's FT/out matmuls are emitted after
    batch b+1's Gram so the PE never waits on the XBAR transposes.
Sharding: pure data parallel, batch dim 64 -> 8 batches per NeuronCore.
"""

import sys
import numpy as np
import ml_dtypes

for _p in ("/opt/trn_rl_repo",):
    if _p not in sys.path:
        sys.path.append(_p)

B, SQ, SKV, DQ, DKV = 64, 8, 4096, 256, 512
NCORES = 8
BL = B // NCORES          # local batches per core
R = BL * SQ               # query rows per core
LN_EPS = 1e-5
SCALE = 1.0 / (DQ ** 0.5)

_CACHE = {}

# mirror blocks of the symmetric G: (dst_row, dst_col, src_row, src_col)
MIRRORS = ((1, 0, 0, 1), (2, 0, 0, 2), (2, 1, 1, 2),
           (3, 0, 0, 3), (3, 1, 1, 3), (3, 2, 2, 3))


def _build():
    import concourse.mybir as mybir
    import concourse.tile as tile
    from concourse import bacc
    from concourse.masks import make_identity

    dt = mybir.dt
    f32, bf16, f8 = dt.float32, dt.bfloat16, dt.float8e4
    DR = mybir.MatmulPerfMode.DoubleRow
    NSUB = SKV // 128    # 32 s-subchunks per batch
    NPAIR = NSUB // 2    # 16 DoubleRow pairs per batch

    nc = bacc.Bacc("TRN2", target_bir_lowering=False, debug=False,
                   num_devices=NCORES)
    q_d = nc.dram_tensor("q", (R, DQ), f32, kind="ExternalInput")
    qs_d = nc.dram_tensor("qsw", (R, DQ), f32, kind="ExternalInput")
    kv_d = nc.dram_tensor("kv", (BL, SKV, DKV), f8, kind="ExternalInput")
    c_d = nc.dram_tensor("cosf", (R, DQ), f32, kind="ExternalInput")
    s_d = nc.dram_tensor("sinf", (R, DQ), f32, kind="ExternalInput")
    wk_d = nc.dram_tensor("wk", (128, 2, DKV), bf16, kind="ExternalInput")
    bk_d = nc.dram_tensor("bk", (128, 4), f32, kind="ExternalInput")
    wv_d = nc.dram_tensor("wv", (128, 4, DQ), bf16, kind="ExternalInput")
    out_d = nc.dram_tensor("out", (R, DQ), f32, kind="ExternalOutput")

    with tile.TileContext(nc) as tc:
        with (
            tc.tile_pool(name="const", bufs=1) as const,
            tc.tile_pool(name="qstage", bufs=1) as qpool,
            tc.tile_pool(name="kv", bufs=BL) as kvpool,
            tc.tile_pool(name="gsb", bufs=2) as gpool,
            tc.tile_pool(name="small", bufs=2) as spool,
            tc.tile_pool(name="psg", bufs=2, space="PSUM") as psg,
            tc.tile_pool(name="pss", bufs=2, space="PSUM") as pss,
        ):
            # ---- kv DMAs: all batches SBUF-resident, issued on sync ----
            # batch 0 in 8 fine pieces so the first Gram pair can start
            # as soon as ~256KB has landed; later batches in 4 pieces.
            kv_tiles = []
            for b in range(BL):
                kv_b = kv_d[b].rearrange("(p n) d -> p n d", p=128)
                kvt = kvpool.tile([128, NSUB, DKV], f8, tag="kvt")
                npiece = 8 if b == 0 else 4
                step = NSUB // npiece
                for p in range(npiece):
                    nc.sync.dma_start(kvt[:, p * step:(p + 1) * step, :],
                                      kv_b[:, p * step:(p + 1) * step, :])
                kv_tiles.append(kvt)

            # weights on the scalar (Activation) DGE queue
            wk_sb = const.tile([128, 2, DKV], bf16)
            nc.scalar.dma_start(wk_sb[:], wk_d[:])
            bk_sb = const.tile([128, 4], f32)
            nc.scalar.dma_start(bk_sb[:], bk_d[:])
            wv_sb = const.tile([128, 4, DQ], bf16)
            nc.scalar.dma_start(wv_sb[:], wv_d[:])

            # q stage inputs on the gpsimd (software) DGE queue
            q_sb = qpool.tile([R, DQ], f32)
            nc.gpsimd.dma_start(q_sb[:], q_d[:])
            qs_sb = qpool.tile([R, DQ], f32)
            nc.gpsimd.dma_start(qs_sb[:], qs_d[:])
            c_sb = qpool.tile([R, DQ], f32)
            nc.gpsimd.dma_start(c_sb[:], c_d[:])
            s_sb = qpool.tile([R, DQ], f32)
            nc.gpsimd.dma_start(s_sb[:], s_d[:])

            ident = const.tile([128, 128], bf16)
            make_identity(nc, ident[:])

            # ---- q: rope + LayerNorm on DVE (all 64 rows at once) ----
            qrot = qpool.tile([R, DQ], f32)
            m2 = qpool.tile([R, DQ], f32)
            nc.vector.tensor_mul(qrot[:], q_sb[:], c_sb[:])
            nc.vector.tensor_mul(m2[:], qs_sb[:], s_sb[:])
            nc.vector.tensor_add(qrot[:], qrot[:], m2[:])

            rsum = qpool.tile([R, 1], f32)
            nc.vector.tensor_reduce(rsum[:], qrot[:], mybir.AxisListType.X,
                                    mybir.AluOpType.add)
            mu = qpool.tile([R, 1], f32)
            nc.vector.tensor_scalar_mul(mu[:], rsum[:], 1.0 / DQ)
            xm = qpool.tile([R, DQ], f32)
            nc.vector.tensor_scalar_sub(xm[:], qrot[:], mu[:])
            sq = qpool.tile([R, DQ], f32)
            nc.vector.tensor_mul(sq[:], xm[:], xm[:])
            vsum = qpool.tile([R, 1], f32)
            nc.vector.tensor_reduce(vsum[:], sq[:], mybir.AxisListType.X,
                                    mybir.AluOpType.add)
            eps = qpool.tile([R, 1], f32)
            nc.gpsimd.memset(eps[:], LN_EPS)
            std = qpool.tile([R, 1], f32)
            nc.scalar.activation(std[:], vsum[:],
                                 mybir.ActivationFunctionType.Sqrt,
                                 bias=eps[:], scale=1.0 / DQ)
            rstd = qpool.tile([R, 1], f32)
            nc.vector.reciprocal(rstd[:], std[:])
            qhat = qpool.tile([R, DQ], bf16)
            nc.vector.tensor_scalar_mul(qhat[:], xm[:], rstd[:])

            # q-projection PE work (qhatT + qkT) is emitted inside the
            # b==0 loop below, after batch 0's Gram matmuls: the PE then
            # starts on Gram as soon as kv lands instead of stalling on
            # the LN chain. It is ready well before batch 0's FT needs it.
            qhatT = const.tile([128, 2, R], bf16)
            qkT = const.tile([128, 4, R], bf16)

            def emit_qproj():
                for ch in range(2):
                    tps = pss.tile([128, R], bf16, tag="scratch")
                    nc.tensor.transpose(tps[:], qhat[:, 128 * ch:128 * (ch + 1)],
                                        ident[:R, :R])
                    nc.vector.tensor_copy(qhatT[:, ch, :], tps[:])
                for j in range(4):
                    ps = pss.tile([128, R], f32, tag="scratch")
                    for ch in range(2):
                        nc.tensor.matmul(ps[:], wk_sb[:, ch, 128 * j:128 * (j + 1)],
                                         qhatT[:, ch, :],
                                         start=(ch == 0), stop=(ch == 1))
                    nc.vector.tensor_scalar_add(qkT[:, j, :], ps[:],
                                                bk_sb[:, j:j + 1])

            # ---- per-batch: Gram triangle + (pipelined) FT/out ----
            out_all = const.tile([SQ, BL, DQ], f32)
            gsb_tiles = {}

            def emit_gram(b):
                kvt = kv_tiles[b]
                ga = psg.tile([128, 512], f32, tag="GA")
                gb = psg.tile([128, 512], f32, tag="GB")
                gc = psg.tile([128, 256], f32, tag="GC")
                for pr in range(NPAIR):
                    nn = 2 * pr
                    st, sp = (pr == 0), (pr == NPAIR - 1)
                    # DoubleRow: operands are [128, 2, W] pairs of
                    # k-subtiles; one instruction accumulates both.
                    nc.tensor.matmul(ga[:, :], kvt[:, nn:nn + 2, 0:128],
                                     kvt[:, nn:nn + 2, 0:512],
                                     start=st, stop=sp, perf_mode=DR)
                    # gb holds two disjoint regions (d1=1 rows at 0:384,
                    # d1=3 rows at 384:512) in one PSUM bank: only the
                    # first mm into the bank carries start, only the
                    # last one stop (pending-zero bits cover the rest).
                    nc.tensor.matmul(gb[:, 0:384], kvt[:, nn:nn + 2, 128:256],
                                     kvt[:, nn:nn + 2, 128:512],
                                     start=st, stop=False, perf_mode=DR)
                    nc.tensor.matmul(gc[:, 0:256], kvt[:, nn:nn + 2, 256:384],
                                     kvt[:, nn:nn + 2, 256:512],
                                     start=st, stop=sp, perf_mode=DR)
                    nc.tensor.matmul(gb[:, 384:512], kvt[:, nn:nn + 2, 384:512],
                                     kvt[:, nn:nn + 2, 384:512],
                                     start=False, stop=sp, perf_mode=DR)

                # upper blocks PSUM -> SBUF (DVE), then mirrors via XBAR
                # DMA transposes split over the two hwdge queues.
                gsb = gpool.tile([128, 4, DKV], bf16, tag="gsb")
                nc.vector.tensor_copy(gsb[:, 0, :], ga[:, :])
                nc.vector.tensor_copy(gsb[:, 1, 128:512], gb[:, 0:384])
                nc.vector.tensor_copy(gsb[:, 2, 256:512], gc[:, 0:256])
                nc.vector.tensor_copy(gsb[:, 3, 384:512], gb[:, 384:512])
                for k, (row, col, sr, sc) in enumerate(MIRRORS):
                    eng = nc.sync if k % 2 == 0 else nc.scalar
                    eng.dma_start_transpose(
                        gsb[:, row, 128 * col:128 * (col + 1)],
                        gsb[:, sr, 128 * sc:128 * (sc + 1)])
                gsb_tiles[b] = gsb

            def emit_ftout(b):
                gsb = gsb_tiles.pop(b)
                # FT[d2, i] = sum_d1 G[d1, d2] qkT[d1, i]
                ftp = pss.tile([128, 4, SQ], f32, tag="scratch")
                for j in range(4):
                    for k in range(4):
                        nc.tensor.matmul(ftp[:, j, :],
                                         gsb[:, k, 128 * j:128 * (j + 1)],
                                         qkT[:, k, SQ * b:SQ * (b + 1)],
                                         start=(k == 0), stop=(k == 3))
                ft_sb = spool.tile([128, 4, SQ], bf16, tag="ft")
                nc.vector.tensor_copy(ft_sb[:], ftp[:])

                # out[i, q] = sum_dk FT[dk, i] WvT[dk, q]
                outp = pss.tile([SQ, DQ], f32, tag="scratch")
                for j in range(4):
                    nc.tensor.matmul(outp[:], ft_sb[:, j, :], wv_sb[:, j, :],
                                     start=(j == 0), stop=(j == 3))
                nc.vector.tensor_copy(out_all[:, b, :], outp[:])
                nc.scalar.dma_start(out_d[SQ * b:SQ * (b + 1), :],
                                    out_all[:, b, :])

            for b in range(BL):
                emit_gram(b)
                if b == 0:
                    emit_qproj()
                if b >= 1:
                    emit_ftout(b - 1)
            emit_ftout(BL - 1)

    nc.compile()
    return nc


def _get_nc():
    if "nc" not in _CACHE:
        _CACHE["nc"] = _build()
    return _CACHE["nc"]


def _prep_inputs(q_real, kv, freq_cos, freq_sin, ln_w, ln_b, Wk, Wv):
    f32 = np.float32
    bf16 = ml_dtypes.bfloat16
    f8 = ml_dtypes.float8_e4m3
    q_real = np.asarray(q_real, f32)
    kv = np.asarray(kv, f32)
    freq_cos = np.asarray(freq_cos, f32)
    freq_sin = np.asarray(freq_sin, f32)
    ln_w = np.asarray(ln_w, f32)
    ln_b = np.asarray(ln_b, f32)
    Wk = np.asarray(Wk, f32)
    Wv = np.asarray(Wv, f32)

    # interleaved cos/sin patterns with rotation signs folded in
    C = np.empty((SQ, DQ), f32)
    C[:, 0::2] = freq_cos
    C[:, 1::2] = freq_cos
    S = np.empty((SQ, DQ), f32)
    S[:, 0::2] = -freq_sin
    S[:, 1::2] = freq_sin
    C = np.tile(C, (BL, 1))
    S = np.tile(S, (BL, 1))

    # pair-swapped q (pure layout shuffle; rotation math runs on device)
    qsw = np.empty_like(q_real)
    qsw[..., 0::2] = q_real[..., 1::2]
    qsw[..., 1::2] = q_real[..., 0::2]

    # fold ln_w and the 1/sqrt(dq) score scale into Wk; ln_b becomes a bias
    wk_f = (ln_w[:, None] * Wk) * SCALE           # (256, 512)
    bk = (ln_b @ Wk) * SCALE                      # (512,)
    wk_arr = np.ascontiguousarray(
        wk_f.reshape(2, 128, DKV).transpose(1, 0, 2)).astype(bf16)
    bk_arr = np.ascontiguousarray(bk.reshape(4, 128).T).astype(f32)
    wv_arr = np.ascontiguousarray(
        Wv.T.reshape(4, 128, DQ).transpose(1, 0, 2)).astype(bf16)

    kv_q = kv.astype(f8)

    in_maps = []
    for c in range(NCORES):
        sl = slice(BL * c, BL * (c + 1))
        in_maps.append({
            "q": np.ascontiguousarray(q_real[sl].reshape(R, DQ)),
            "qsw": np.ascontiguousarray(qsw[sl].reshape(R, DQ)),
            "kv": np.ascontiguousarray(kv_q[sl]),
            "cosf": C,
            "sinf": S,
            "wk": wk_arr,
            "bk": bk_arr,
            "wv": wv_arr,
        })
    return in_maps


def kernel(**inputs):
    from concourse.bass_utils import run_bass_kernel_spmd

    nc = _get_nc()
    in_maps = _prep_inputs(**inputs)
    res = run_bass_kernel_spmd(nc, in_maps, list(range(NCORES)))
    out = np.concatenate(
        [res.results[c]["out"].reshape(BL, SQ, DQ) for c in range(NCORES)], axis=0)
    return np.ascontiguousarray(out.astype(np.float32))


# revision 11
# speedup vs baseline: 1.3825x; 1.3825x over previous
"""Trainium2 Bass kernel for CrossAttentionComplexQ.

Shapes: q_real (64, 8, 256), kv (64, 4096, 512) -> out (64, 8, 256).

Math (per batch b):
    q  = complex-rotate(q_real, freq)           # rope-style pairwise rotation
    qn = LayerNorm(q) * ln_w + ln_b
    k  = kv @ Wk.T ; v = kv @ Wv.T
    out = ((qn @ k.T) / 16) @ v
Since there is no softmax the whole chain is linear in kv, so per batch:
    out = qk' @ G @ Wv.T    with qk' = qn @ Wk / 16 (8, 512)
                            and  G = kv.T @ kv      (512, 512 Gram matrix)
G only ever contracts over the sequence dim s, so kv is consumed in its
natural [s-partition, d-free] SBUF layout -- no transposes of the big
tensor. G is symmetric: only the 10 upper-triangle 128x128 blocks are
computed; mirror blocks come from XBAR DMA transposes (off the PE).

Perf structure:
  - kv cast to fp8 e4m3 on host; Gram matmuls run DoubleRow perf mode
    (k=256 per instruction). PSUM-drain-limited at ~1 out-col/cycle.
  - DMA issue cost (~0.6us/DMA on an engine queue) is spread over the
    three DMA-capable queues: kv on sync, weights + mirror-transposes +
    out on scalar(+sync), q-stage on gpsimd.
  - Software pipelining: batch b's FT/out matmuls are emitted after
    batch b+1's Gram so the PE never waits on the XBAR transposes.
Sharding: pure data parallel, batch dim 64 -> 8 batches per NeuronCore.
"""

import sys
import numpy as np
import ml_dtypes

for _p in ("/opt/trn_rl_repo",):
    if _p not in sys.path:
        sys.path.append(_p)

B, SQ, SKV, DQ, DKV = 64, 8, 4096, 256, 512
NCORES = 8
BL = B // NCORES          # local batches per core
R = BL * SQ               # query rows per core
LN_EPS = 1e-5
SCALE = 1.0 / (DQ ** 0.5)

_CACHE = {}

# mirror blocks of the symmetric G: (dst_row, dst_col, src_row, src_col)
MIRRORS = ((1, 0, 0, 1), (2, 0, 0, 2), (2, 1, 1, 2),
           (3, 0, 0, 3), (3, 1, 1, 3), (3, 2, 2, 3))


def _build():
    import concourse.mybir as mybir
    import concourse.tile as tile
    from concourse import bacc
    from concourse.masks import make_identity

    dt = mybir.dt
    f32, bf16, f8 = dt.float32, dt.bfloat16, dt.float8e4
    DR = mybir.MatmulPerfMode.DoubleRow
    NSUB = SKV // 128    # 32 s-subchunks per batch
    NPAIR = NSUB // 2    # 16 DoubleRow pairs per batch

    nc = bacc.Bacc("TRN2", target_bir_lowering=False, debug=False,
                   num_devices=NCORES)
    q_d = nc.dram_tensor("q", (R, DQ), f32, kind="ExternalInput")
    qs_d = nc.dram_tensor("qsw", (R, DQ), f32, kind="ExternalInput")
    kv_d = nc.dram_tensor("kv", (BL, SKV, DKV), f8, kind="ExternalInput")
    c_d = nc.dram_tensor("cosf", (R, DQ), f32, kind="ExternalInput")
    s_d = nc.dram_tensor("sinf", (R, DQ), f32, kind="ExternalInput")
    wk_d = nc.dram_tensor("wk", (128, 2, DKV), bf16, kind="ExternalInput")
    bk_d = nc.dram_tensor("bk", (128, 4), f32, kind="ExternalInput")
    wv_d = nc.dram_tensor("wv", (128, 4, DQ), bf16, kind="ExternalInput")
    out_d = nc.dram_tensor("out", (R, DQ), f32, kind="ExternalOutput")

    with tile.TileContext(nc) as tc:
        with (
            tc.tile_pool(name="const", bufs=1) as const,
            tc.tile_pool(name="qstage", bufs=1) as qpool,
            tc.tile_pool(name="kv", bufs=BL) as kvpool,
            tc.tile_pool(name="gsb", bufs=2) as gpool,
            tc.tile_pool(name="small", bufs=2) as spool,
            tc.tile_pool(name="psg", bufs=2, space="PSUM") as psg,
            tc.tile_pool(name="pss", bufs=2, space="PSUM") as pss,
        ):
            # ---- kv DMAs: all batches SBUF-resident, issued on sync ----
            # batch 0 in 8 fine pieces so the first Gram pair can start
            # as soon as ~256KB has landed; later batches in 4 pieces.
            kv_tiles = []
            for b in range(BL):
                kv_b = kv_d[b].rearrange("(p n) d -> p n d", p=128)
                kvt = kvpool.tile([128, NSUB, DKV], f8, tag="kvt")
                npiece = 8 if b == 0 else 4
                step = NSUB // npiece
                for p in range(npiece):
                    nc.sync.dma_start(kvt[:, p * step:(p + 1) * step, :],
                                      kv_b[:, p * step:(p + 1) * step, :])
                kv_tiles.append(kvt)

            # weights on the scalar (Activation) DGE queue
            wk_sb = const.tile([128, 2, DKV], bf16)
            nc.scalar.dma_start(wk_sb[:], wk_d[:])
            bk_sb = const.tile([128, 4], f32)
            nc.scalar.dma_start(bk_sb[:], bk_d[:])
            wv_sb = const.tile([128, 4, DQ], bf16)
            nc.scalar.dma_start(wv_sb[:], wv_d[:])

            # q stage inputs on the gpsimd (software) DGE queue
            q_sb = qpool.tile([R, DQ], f32)
            nc.gpsimd.dma_start(q_sb[:], q_d[:])
            qs_sb = qpool.tile([R, DQ], f32)
            nc.gpsimd.dma_start(qs_sb[:], qs_d[:])
            c_sb = qpool.tile([R, DQ], f32)
            nc.gpsimd.dma_start(c_sb[:], c_d[:])
            s_sb = qpool.tile([R, DQ], f32)
            nc.gpsimd.dma_start(s_sb[:], s_d[:])

            ident = const.tile([128, 128], bf16)
            make_identity(nc, ident[:])

            # ---- q: rope + LayerNorm on DVE (all 64 rows at once) ----
            qrot = qpool.tile([R, DQ], f32)
            m2 = qpool.tile([R, DQ], f32)
            nc.vector.tensor_mul(qrot[:], q_sb[:], c_sb[:])
            nc.vector.tensor_mul(m2[:], qs_sb[:], s_sb[:])
            nc.vector.tensor_add(qrot[:], qrot[:], m2[:])

            rsum = qpool.tile([R, 1], f32)
            nc.vector.tensor_reduce(rsum[:], qrot[:], mybir.AxisListType.X,
                                    mybir.AluOpType.add)
            mu = qpool.tile([R, 1], f32)
            nc.vector.tensor_scalar_mul(mu[:], rsum[:], 1.0 / DQ)
            xm = qpool.tile([R, DQ], f32)
            nc.vector.tensor_scalar_sub(xm[:], qrot[:], mu[:])
            sq = qpool.tile([R, DQ], f32)
            nc.vector.tensor_mul(sq[:], xm[:], xm[:])
            vsum = qpool.tile([R, 1], f32)
            nc.vector.tensor_reduce(vsum[:], sq[:], mybir.AxisListType.X,
                                    mybir.AluOpType.add)
            eps = qpool.tile([R, 1], f32)
            nc.gpsimd.memset(eps[:], LN_EPS)
            std = qpool.tile([R, 1], f32)
            nc.scalar.activation(std[:], vsum[:],
                                 mybir.ActivationFunctionType.Sqrt,
                                 bias=eps[:], scale=1.0 / DQ)
            rstd = qpool.tile([R, 1], f32)
            nc.vector.reciprocal(rstd[:], std[:])
            qhat = qpool.tile([R, DQ], bf16)
            nc.vector.tensor_scalar_mul(qhat[:], xm[:], rstd[:])

            # q-projection PE work (qhatT + qkT) is emitted inside the
            # b==0 loop below, after batch 0's Gram matmuls: the PE then
            # starts on Gram as soon as kv lands instead of stalling on
            # the LN chain. It is ready well before batch 0's FT needs it.
            qhatT = const.tile([128, 2, R], bf16)
            qkT = const.tile([128, 4, R], bf16)

            def emit_qproj():
                for ch in range(2):
                    tps = pss.tile([128, R], bf16, tag="scratch")
                    nc.tensor.transpose(tps[:], qhat[:, 128 * ch:128 * (ch + 1)],
                                        ident[:R, :R])
                    nc.vector.tensor_copy(qhatT[:, ch, :], tps[:])
                for j in range(4):
                    ps = pss.tile([128, R], f32, tag="scratch")
                    for ch in range(2):
                        nc.tensor.matmul(ps[:], wk_sb[:, ch, 128 * j:128 * (j + 1)],
                                         qhatT[:, ch, :],
                                         start=(ch == 0), stop=(ch == 1))
                    nc.vector.tensor_scalar_add(qkT[:, j, :], ps[:],
                                                bk_sb[:, j:j + 1])

            # ---- per-batch: Gram triangle + (pipelined) FT/out ----
            out_all = const.tile([SQ, BL, DQ], f32)
            gsb_tiles = {}

            def emit_gram(b):
                kvt = kv_tiles[b]
                ga = psg.tile([128, 512], f32, tag="GA")
                gb = psg.tile([128, 512], f32, tag="GB")
                gc = psg.tile([128, 256], f32, tag="GC")
                for pr in range(NPAIR):
                    nn = 2 * pr
                    st, sp = (pr == 0), (pr == NPAIR - 1)
                    # DoubleRow: operands are [128, 2, W] pairs of
                    # k-subtiles; one instruction accumulates both.
                    nc.tensor.matmul(ga[:, :], kvt[:, nn:nn + 2, 0:128],
                                     kvt[:, nn:nn + 2, 0:512],
                                     start=st, stop=sp, perf_mode=DR)
                    # gb holds two disjoint regions (d1=1 rows at 0:384,
                    # d1=3 rows at 384:512) in one PSUM bank: only the
                    # first mm into the bank carries start, only the
                    # last one stop (pending-zero bits cover the rest).
                    nc.tensor.matmul(gb[:, 0:384], kvt[:, nn:nn + 2, 128:256],
                                     kvt[:, nn:nn + 2, 128:512],
                                     start=st, stop=False, perf_mode=DR)
                    nc.tensor.matmul(gc[:, 0:256], kvt[:, nn:nn + 2, 256:384],
                                     kvt[:, nn:nn + 2, 256:512],
                                     start=st, stop=sp, perf_mode=DR)
                    nc.tensor.matmul(gb[:, 384:512], kvt[:, nn:nn + 2, 384:512],
                                     kvt[:, nn:nn + 2, 384:512],
                                     start=False, stop=sp, perf_mode=DR)

                # upper blocks PSUM -> SBUF (DVE); mirrors are emitted in
                # emit_ftout one batch later so the casts have a full Gram
                # of slack before the PE touches them.
                gsb = gpool.tile([128, 4, DKV], bf16, tag="gsb")
                nc.vector.tensor_copy(gsb[:, 0, :], ga[:, :])
                nc.vector.tensor_copy(gsb[:, 1, 128:512], gb[:, 0:384])
                nc.vector.tensor_copy(gsb[:, 2, 256:512], gc[:, 0:256])
                nc.vector.tensor_copy(gsb[:, 3, 384:512], gb[:, 384:512])
                gsb_tiles[b] = gsb

            def emit_ftout(b):
                gsb = gsb_tiles.pop(b)
                # mirror blocks via PE transpose (G symmetric)
                for (row, col, sr, sc) in MIRRORS:
                    tps = pss.tile([128, 128], bf16, tag="scratch")
                    nc.tensor.transpose(
                        tps[:], gsb[:, sr, 128 * sc:128 * (sc + 1)], ident[:])
                    nc.vector.tensor_copy(
                        gsb[:, row, 128 * col:128 * (col + 1)], tps[:])
                # FT[d2, i] = sum_d1 G[d1, d2] qkT[d1, i]
                ftp = pss.tile([128, 4, SQ], f32, tag="scratch")
                for j in range(4):
                    for k in range(4):
                        nc.tensor.matmul(ftp[:, j, :],
                                         gsb[:, k, 128 * j:128 * (j + 1)],
                                         qkT[:, k, SQ * b:SQ * (b + 1)],
                                         start=(k == 0), stop=(k == 3))
                ft_sb = spool.tile([128, 4, SQ], bf16, tag="ft")
                nc.vector.tensor_copy(ft_sb[:], ftp[:])

                # out[i, q] = sum_dk FT[dk, i] WvT[dk, q]
                outp = pss.tile([SQ, DQ], f32, tag="scratch")
                for j in range(4):
                    nc.tensor.matmul(outp[:], ft_sb[:, j, :], wv_sb[:, j, :],
                                     start=(j == 0), stop=(j == 3))
                nc.vector.tensor_copy(out_all[:, b, :], outp[:])
                nc.scalar.dma_start(out_d[SQ * b:SQ * (b + 1), :],
                                    out_all[:, b, :])

            for b in range(BL):
                emit_gram(b)
                if b == 0:
                    emit_qproj()
                if b >= 1:
                    emit_ftout(b - 1)
            emit_ftout(BL - 1)

    nc.compile()
    return nc


def _get_nc():
    if "nc" not in _CACHE:
        _CACHE["nc"] = _build()
    return _CACHE["nc"]


def _prep_inputs(q_real, kv, freq_cos, freq_sin, ln_w, ln_b, Wk, Wv):
    f32 = np.float32
    bf16 = ml_dtypes.bfloat16
    f8 = ml_dtypes.float8_e4m3
    q_real = np.asarray(q_real, f32)
    kv = np.asarray(kv, f32)
    freq_cos = np.asarray(freq_cos, f32)
    freq_sin = np.asarray(freq_sin, f32)
    ln_w = np.asarray(ln_w, f32)
    ln_b = np.asarray(ln_b, f32)
    Wk = np.asarray(Wk, f32)
    Wv = np.asarray(Wv, f32)

    # interleaved cos/sin patterns with rotation signs folded in
    C = np.empty((SQ, DQ), f32)
    C[:, 0::2] = freq_cos
    C[:, 1::2] = freq_cos
    S = np.empty((SQ, DQ), f32)
    S[:, 0::2] = -freq_sin
    S[:, 1::2] = freq_sin
    C = np.tile(C, (BL, 1))
    S = np.tile(S, (BL, 1))

    # pair-swapped q (pure layout shuffle; rotation math runs on device)
    qsw = np.empty_like(q_real)
    qsw[..., 0::2] = q_real[..., 1::2]
    qsw[..., 1::2] = q_real[..., 0::2]

    # fold ln_w and the 1/sqrt(dq) score scale into Wk; ln_b becomes a bias
    wk_f = (ln_w[:, None] * Wk) * SCALE           # (256, 512)
    bk = (ln_b @ Wk) * SCALE                      # (512,)
    wk_arr = np.ascontiguousarray(
        wk_f.reshape(2, 128, DKV).transpose(1, 0, 2)).astype(bf16)
    bk_arr = np.ascontiguousarray(bk.reshape(4, 128).T).astype(f32)
    wv_arr = np.ascontiguousarray(
        Wv.T.reshape(4, 128, DQ).transpose(1, 0, 2)).astype(bf16)

    kv_q = kv.astype(f8)

    in_maps = []
    for c in range(NCORES):
        sl = slice(BL * c, BL * (c + 1))
        in_maps.append({
            "q": np.ascontiguousarray(q_real[sl].reshape(R, DQ)),
            "qsw": np.ascontiguousarray(qsw[sl].reshape(R, DQ)),
            "kv": np.ascontiguousarray(kv_q[sl]),
            "cosf": C,
            "sinf": S,
            "wk": wk_arr,
            "bk": bk_arr,
            "wv": wv_arr,
        })
    return in_maps


def kernel(**inputs):
    from concourse.bass_utils import run_bass_kernel_spmd

    nc = _get_nc()
    in_maps = _prep_inputs(**inputs)
    res = run_bass_kernel_spmd(nc, in_maps, list(range(NCORES)))
    out = np.concatenate(
        [res.results[c]["out"].reshape(BL, SQ, DQ) for c in range(NCORES)], axis=0)
    return np.ascontiguousarray(out.astype(np.float32))


# revision 15
# speedup vs baseline: 1.4240x; 1.0300x over previous
"""Trainium2 Bass kernel for CrossAttentionComplexQ.

Shapes: q_real (64, 8, 256), kv (64, 4096, 512) -> out (64, 8, 256).

Math (per batch b):
    q  = complex-rotate(q_real, freq)           # rope-style pairwise rotation
    qn = LayerNorm(q) * ln_w + ln_b
    k  = kv @ Wk.T ; v = kv @ Wv.T
    out = ((qn @ k.T) / 16) @ v
Since there is no softmax the whole chain is linear in kv, so per batch:
    out = qk' @ G @ Wv.T    with qk' = qn @ Wk / 16 (8, 512)
                            and  G = kv.T @ kv      (512, 512 Gram matrix)
G only ever contracts over the sequence dim s, so kv is consumed in its
natural [s-partition, d-free] SBUF layout -- no transposes of the big
tensor. G is symmetric: only the 10 upper-triangle 128x128 blocks are
computed; mirror blocks come from XBAR DMA transposes (off the PE).

Perf structure:
  - kv cast to fp8 e4m3 on host; Gram matmuls run DoubleRow perf mode
    (k=256 per instruction). PSUM-drain-limited at ~1 out-col/cycle.
  - DMA issue cost (~0.6us/DMA on an engine queue) is spread over the
    three DMA-capable queues: kv on sync, weights + mirror-transposes +
    out on scalar(+sync), q-stage on gpsimd.
  - Software pipelining: batch b's FT/out matmuls are emitted after
    batch b+1's Gram so the PE never waits on the XBAR transposes.
Sharding: pure data parallel, batch dim 64 -> 8 batches per NeuronCore.
"""

import sys
import numpy as np
import ml_dtypes

for _p in ("/opt/trn_rl_repo",):
    if _p not in sys.path:
        sys.path.append(_p)

B, SQ, SKV, DQ, DKV = 64, 8, 4096, 256, 512
NCORES = 8
BL = B // NCORES          # local batches per core
R = BL * SQ               # query rows per core
LN_EPS = 1e-5
SCALE = 1.0 / (DQ ** 0.5)

_CACHE = {}

# mirror blocks of the symmetric G: (dst_row, dst_col, src_row, src_col)
MIRRORS = ((1, 0, 0, 1), (2, 0, 0, 2), (2, 1, 1, 2),
           (3, 0, 0, 3), (3, 1, 1, 3), (3, 2, 2, 3))


def _build():
    import concourse.mybir as mybir
    import concourse.tile as tile
    from concourse import bacc
    from concourse.masks import make_identity

    dt = mybir.dt
    f32, bf16, f8 = dt.float32, dt.bfloat16, dt.float8e4
    DR = mybir.MatmulPerfMode.DoubleRow
    NSUB = SKV // 128    # 32 s-subchunks per batch
    NPAIR = NSUB // 2    # 16 DoubleRow pairs per batch

    nc = bacc.Bacc("TRN2", target_bir_lowering=False, debug=False,
                   num_devices=NCORES)
    q_d = nc.dram_tensor("q", (R, DQ), f32, kind="ExternalInput")
    qs_d = nc.dram_tensor("qsw", (R, DQ), f32, kind="ExternalInput")
    kv_d = nc.dram_tensor("kv", (BL, SKV, DKV), f8, kind="ExternalInput")
    c_d = nc.dram_tensor("cosf", (R, DQ), f32, kind="ExternalInput")
    s_d = nc.dram_tensor("sinf", (R, DQ), f32, kind="ExternalInput")
    wk_d = nc.dram_tensor("wk", (128, 2, DKV), bf16, kind="ExternalInput")
    bk_d = nc.dram_tensor("bk", (128, 4), f32, kind="ExternalInput")
    wv_d = nc.dram_tensor("wv", (128, 4, DQ), bf16, kind="ExternalInput")
    out_d = nc.dram_tensor("out", (R, DQ), f32, kind="ExternalOutput")

    with tile.TileContext(nc) as tc:
        with (
            tc.tile_pool(name="const", bufs=1) as const,
            tc.tile_pool(name="qstage", bufs=1) as qpool,
            tc.tile_pool(name="kv", bufs=1) as kvpool,
            tc.tile_pool(name="gsb", bufs=2) as gpool,
            tc.tile_pool(name="small", bufs=2) as spool,
            tc.tile_pool(name="psg", bufs=2, space="PSUM") as psg,
            tc.tile_pool(name="pss", bufs=2, space="PSUM") as pss,
        ):
            # ---- kv DMAs: all batches SBUF-resident, issued on sync ----
            # Dependency tracking gates a consumer on the WHOLE tile, so
            # each DMA piece gets its own tile: batch 0 in 8 fine tiles
            # (first Gram pair starts after ~256KB), later batches in 4.
            kv_tiles = {}          # (b, subchunk) -> (tile, local subchunk)
            for b in range(BL):
                kv_b = kv_d[b].rearrange("(p n) d -> p n d", p=128)
                npiece = 8 if b == 0 else 4
                step = NSUB // npiece
                for p in range(npiece):
                    kvt = kvpool.tile([128, step, DKV], f8,
                                      tag=f"kv_b{b}_p{p}")
                    nc.sync.dma_start(kvt[:],
                                      kv_b[:, p * step:(p + 1) * step, :])
                    for nn in range(step):
                        kv_tiles[(b, p * step + nn)] = (kvt, nn)

            # weights on the scalar (Activation) DGE queue
            wk_sb = const.tile([128, 2, DKV], bf16)
            nc.scalar.dma_start(wk_sb[:], wk_d[:])
            bk_sb = const.tile([128, 4], f32)
            nc.scalar.dma_start(bk_sb[:], bk_d[:])
            wv_sb = const.tile([128, 4, DQ], bf16)
            nc.scalar.dma_start(wv_sb[:], wv_d[:])

            # q stage inputs on the gpsimd (software) DGE queue
            q_sb = qpool.tile([R, DQ], f32)
            nc.gpsimd.dma_start(q_sb[:], q_d[:])
            qs_sb = qpool.tile([R, DQ], f32)
            nc.gpsimd.dma_start(qs_sb[:], qs_d[:])
            c_sb = qpool.tile([R, DQ], f32)
            nc.gpsimd.dma_start(c_sb[:], c_d[:])
            s_sb = qpool.tile([R, DQ], f32)
            nc.gpsimd.dma_start(s_sb[:], s_d[:])

            ident = const.tile([128, 128], bf16)
            make_identity(nc, ident[:])

            # ---- q: rope + LayerNorm on DVE (all 64 rows at once) ----
            qrot = qpool.tile([R, DQ], f32)
            m2 = qpool.tile([R, DQ], f32)
            nc.vector.tensor_mul(qrot[:], q_sb[:], c_sb[:])
            nc.vector.tensor_mul(m2[:], qs_sb[:], s_sb[:])
            nc.vector.tensor_add(qrot[:], qrot[:], m2[:])

            rsum = qpool.tile([R, 1], f32)
            nc.vector.tensor_reduce(rsum[:], qrot[:], mybir.AxisListType.X,
                                    mybir.AluOpType.add)
            mu = qpool.tile([R, 1], f32)
            nc.vector.tensor_scalar_mul(mu[:], rsum[:], 1.0 / DQ)
            xm = qpool.tile([R, DQ], f32)
            nc.vector.tensor_scalar_sub(xm[:], qrot[:], mu[:])
            sq = qpool.tile([R, DQ], f32)
            nc.vector.tensor_mul(sq[:], xm[:], xm[:])
            vsum = qpool.tile([R, 1], f32)
            nc.vector.tensor_reduce(vsum[:], sq[:], mybir.AxisListType.X,
                                    mybir.AluOpType.add)
            eps = qpool.tile([R, 1], f32)
            nc.gpsimd.memset(eps[:], LN_EPS)
            std = qpool.tile([R, 1], f32)
            nc.scalar.activation(std[:], vsum[:],
                                 mybir.ActivationFunctionType.Sqrt,
                                 bias=eps[:], scale=1.0 / DQ)
            rstd = qpool.tile([R, 1], f32)
            nc.vector.reciprocal(rstd[:], std[:])
            qhat = qpool.tile([R, DQ], bf16)
            nc.vector.tensor_scalar_mul(qhat[:], xm[:], rstd[:])

            # q-projection PE work (qhatT + qkT) is emitted inside the
            # b==0 loop below, after batch 0's Gram matmuls: the PE then
            # starts on Gram as soon as kv lands instead of stalling on
            # the LN chain. It is ready well before batch 0's FT needs it.
            qhatT = const.tile([128, 2, R], bf16)
            qkT = const.tile([128, 4, R], bf16)

            def emit_qproj():
                for ch in range(2):
                    tps = pss.tile([128, R], bf16, tag="scratch")
                    nc.tensor.transpose(tps[:], qhat[:, 128 * ch:128 * (ch + 1)],
                                        ident[:R, :R])
                    nc.vector.tensor_copy(qhatT[:, ch, :], tps[:])
                for j in range(4):
                    ps = pss.tile([128, R], f32, tag="scratch")
                    for ch in range(2):
                        nc.tensor.matmul(ps[:], wk_sb[:, ch, 128 * j:128 * (j + 1)],
                                         qhatT[:, ch, :],
                                         start=(ch == 0), stop=(ch == 1))
                    nc.vector.tensor_scalar_add(qkT[:, j, :], ps[:],
                                                bk_sb[:, j:j + 1])

            # ---- per-batch: Gram triangle + (pipelined) FT/out ----
            out_all = const.tile([SQ, BL, DQ], f32)
            gsb_tiles = {}

            def emit_gram(b):
                ga = psg.tile([128, 512], f32, tag="GA")
                gb = psg.tile([128, 512], f32, tag="GB")
                gc = psg.tile([128, 256], f32, tag="GC")
                for pr in range(NPAIR):
                    kvt, nn = kv_tiles[(b, 2 * pr)]
                    st, sp = (pr == 0), (pr == NPAIR - 1)
                    # DoubleRow: operands are [128, 2, W] pairs of
                    # k-subtiles; one instruction accumulates both.
                    nc.tensor.matmul(ga[:, :], kvt[:, nn:nn + 2, 0:128],
                                     kvt[:, nn:nn + 2, 0:512],
                                     start=st, stop=sp, perf_mode=DR)
                    # gb holds two disjoint regions (d1=1 rows at 0:384,
                    # d1=3 rows at 384:512) in one PSUM bank: only the
                    # first mm into the bank carries start, only the
                    # last one stop (pending-zero bits cover the rest).
                    nc.tensor.matmul(gb[:, 0:384], kvt[:, nn:nn + 2, 128:256],
                                     kvt[:, nn:nn + 2, 128:512],
                                     start=st, stop=False, perf_mode=DR)
                    nc.tensor.matmul(gc[:, 0:256], kvt[:, nn:nn + 2, 256:384],
                                     kvt[:, nn:nn + 2, 256:512],
                                     start=st, stop=sp, perf_mode=DR)
                    nc.tensor.matmul(gb[:, 384:512], kvt[:, nn:nn + 2, 384:512],
                                     kvt[:, nn:nn + 2, 384:512],
                                     start=False, stop=sp, perf_mode=DR)

                # upper blocks PSUM -> SBUF (DVE); mirrors are emitted in
                # emit_ftout one batch later so the casts have a full Gram
                # of slack before the PE touches them.
                gsb = gpool.tile([128, 4, DKV], bf16, tag="gsb")
                nc.vector.tensor_copy(gsb[:, 0, :], ga[:, :])
                nc.vector.tensor_copy(gsb[:, 1, 128:512], gb[:, 0:384])
                nc.vector.tensor_copy(gsb[:, 2, 256:512], gc[:, 0:256])
                nc.vector.tensor_copy(gsb[:, 3, 384:512], gb[:, 384:512])
                gsb_tiles[b] = gsb

            def emit_ftout(b):
                gsb = gsb_tiles.pop(b)
                # mirror blocks via PE transpose (G symmetric)
                for (row, col, sr, sc) in MIRRORS:
                    tps = pss.tile([128, 128], bf16, tag="scratch")
                    nc.tensor.transpose(
                        tps[:], gsb[:, sr, 128 * sc:128 * (sc + 1)], ident[:])
                    nc.vector.tensor_copy(
                        gsb[:, row, 128 * col:128 * (col + 1)], tps[:])
                # FT[d2, i] = sum_d1 G[d1, d2] qkT[d1, i]
                ftp = pss.tile([128, 4, SQ], f32, tag="scratch")
                for j in range(4):
                    for k in range(4):
                        nc.tensor.matmul(ftp[:, j, :],
                                         gsb[:, k, 128 * j:128 * (j + 1)],
                                         qkT[:, k, SQ * b:SQ * (b + 1)],
                                         start=(k == 0), stop=(k == 3))
                ft_sb = spool.tile([128, 4, SQ], bf16, tag="ft")
                nc.vector.tensor_copy(ft_sb[:], ftp[:])

                # out[i, q] = sum_dk FT[dk, i] WvT[dk, q]
                outp = pss.tile([SQ, DQ], f32, tag="scratch")
                for j in range(4):
                    nc.tensor.matmul(outp[:], ft_sb[:, j, :], wv_sb[:, j, :],
                                     start=(j == 0), stop=(j == 3))
                nc.vector.tensor_copy(out_all[:, b, :], outp[:])
                nc.scalar.dma_start(out_d[SQ * b:SQ * (b + 1), :],
                                    out_all[:, b, :])

            for b in range(BL):
                emit_gram(b)
                if b == 0:
                    emit_qproj()
                if b >= 1:
                    emit_ftout(b - 1)
            emit_ftout(BL - 1)

    nc.compile()
    return nc


def _get_nc():
    if "nc" not in _CACHE:
        _CACHE["nc"] = _build()
    return _CACHE["nc"]


def _prep_inputs(q_real, kv, freq_cos, freq_sin, ln_w, ln_b, Wk, Wv):
    f32 = np.float32
    bf16 = ml_dtypes.bfloat16
    f8 = ml_dtypes.float8_e4m3
    q_real = np.asarray(q_real, f32)
    kv = np.asarray(kv, f32)
    freq_cos = np.asarray(freq_cos, f32)
    freq_sin = np.asarray(freq_sin, f32)
    ln_w = np.asarray(ln_w, f32)
    ln_b = np.asarray(ln_b, f32)
    Wk = np.asarray(Wk, f32)
    Wv = np.asarray(Wv, f32)

    # interleaved cos/sin patterns with rotation signs folded in
    C = np.empty((SQ, DQ), f32)
    C[:, 0::2] = freq_cos
    C[:, 1::2] = freq_cos
    S = np.empty((SQ, DQ), f32)
    S[:, 0::2] = -freq_sin
    S[:, 1::2] = freq_sin
    C = np.tile(C, (BL, 1))
    S = np.tile(S, (BL, 1))

    # pair-swapped q (pure layout shuffle; rotation math runs on device)
    qsw = np.empty_like(q_real)
    qsw[..., 0::2] = q_real[..., 1::2]
    qsw[..., 1::2] = q_real[..., 0::2]

    # fold ln_w and the 1/sqrt(dq) score scale into Wk; ln_b becomes a bias
    wk_f = (ln_w[:, None] * Wk) * SCALE           # (256, 512)
    bk = (ln_b @ Wk) * SCALE                      # (512,)
    wk_arr = np.ascontiguousarray(
        wk_f.reshape(2, 128, DKV).transpose(1, 0, 2)).astype(bf16)
    bk_arr = np.ascontiguousarray(bk.reshape(4, 128).T).astype(f32)
    wv_arr = np.ascontiguousarray(
        Wv.T.reshape(4, 128, DQ).transpose(1, 0, 2)).astype(bf16)

    kv_q = kv.astype(f8)

    in_maps = []
    for c in range(NCORES):
        sl = slice(BL * c, BL * (c + 1))
        in_maps.append({
            "q": np.ascontiguousarray(q_real[sl].reshape(R, DQ)),
            "qsw": np.ascontiguousarray(qsw[sl].reshape(R, DQ)),
            "kv": np.ascontiguousarray(kv_q[sl]),
            "cosf": C,
            "sinf": S,
            "wk": wk_arr,
            "bk": bk_arr,
            "wv": wv_arr,
        })
    return in_maps


def kernel(**inputs):
    from concourse.bass_utils import run_bass_kernel_spmd

    nc = _get_nc()
    in_maps = _prep_inputs(**inputs)
    res = run_bass_kernel_spmd(nc, in_maps, list(range(NCORES)))
    out = np.concatenate(
        [res.results[c]["out"].reshape(BL, SQ, DQ) for c in range(NCORES)], axis=0)
    return np.ascontiguousarray(out.astype(np.float32))
